# revision 21
# baseline (speedup 1.0000x reference)
"""Category-specific linear (MoE-style routed batched matmul) on 8 trn2 cores.

out[b, s, h] = sum_i x[b, s, i] * W[cat_ids[b], i, h] + bias[cat_ids[b], h]

Shapes (hardcoded): x (32, 512, 1024) f32, cat_ids (32,) int, W (16, 1024, 4096)
f32, b (16, 4096) f32 -> out (32, 512, 4096) f32.

Strategy: data-parallel over batch, 4 batches per core, with host-side routing
that always packs one same-category PAIR of batches plus two singles per core
(slot capacities [2, 1, 1] batches). With 32 batches over 16 categories there
are always >= (32 - 16)/2 = 8 disjoint same-category pairs, so this packing is
feasible for ANY cat_ids. Each core then loads only 3 weight matrices in f16
(25 MB), keeping HBM traffic below the PE roofline.

Per core (slot-major):
  for slot s in [A(2 batches), B(1), C(1)]:
    for n-half (2 x 2048 cols):
      stream W[s]-half as 8 k-tiles [128, 2048] (512 KB DMAs)
      for m over the slot's 128-sample tiles (8 for A, 4 for B/C):
        for kt(8) x n4(4): f16 matmul -> psum[n4] (accumulate over kt)
        evict psum + bias (DVE add) -> f16 sbuf, DMA to out (scalar ring)

The inner n4 loop keeps 4 matmuls per LDWEIGHTS: that ratio is what lets the
PE's reorder window hide the stationary-operand load (measured: 216 ns/MM
with 4 MMs/LDW vs 259 ns/MM with 2 MMs/LDW -- a quarter-width variant was
7 us/core slower despite better DMA pacing).

Perf structure:
  - xt is pre-transposed ON HOST to partition-major [128, kt, m] (contiguous
    DMA lines; the naive k-major layout cost 21 us of descriptor generation).
    Batch 0 rides the scalar ring; batches 1-3 ride the gpsimd (SWDGE) ring
    so mid-kernel output stores never queue behind them.
  - the first W half is split across the sync AND scalar rings (4 tiles
    each) to double early-delivery rate while the pipe fills; every other
    half streams on the sync ring alone.
  - bias rides to SBUF once as a 48 KB row and is partition-broadcast
    on-chip (gpsimd InstPartitionBroadcast), not re-read 128x from HBM.
  - outputs are stored as f16 (host upcasts); ~2e-4 added error against a
    2e-2 budget, halving the 33.5 MB of output writes.
  - pipe-fill phase: the first half's m-tiles 0-1 run in kt-OUTER order
    (all 8 psum banks, 8 matmuls per arriving W k-tile ~= the HWDGE
    delivery rate), so the PE does useful work while the pipe fills
    instead of stalling mid-m-tile and re-throttling the HAM clock gate.
    A few warmup matmuls cover the first ~2 us before any W has landed.
  - the last m-tile's eviction is split into 4 per-n4 stores (on the by-then
    idle sync ring) so the final HBM write + receipt is 4x smaller.
"""

import numpy as np

import concourse.bacc as bacc
import concourse.mybir as mybir
import concourse.bass as bass
import concourse.tile as tile
from concourse.bass import _add_dep_helper
from concourse.bass_utils import run_bass_kernel_spmd

N_CORES = 8
B, S, K, H = 32, 512, 1024, 4096
BPC = B // N_CORES          # batches per core
P = 128                     # partitions
KT = K // P                 # k tiles (8)
KTC = 2                     # k tiles per xt chunk
NXC = KT // KTC             # xt chunks per batch (4)
MT = S // P                 # sample tiles per batch (4)
NHALF = 2                   # n halves
NH = H // NHALF             # cols per half (2048)
NMM = NH // 512             # 512-wide matmuls per half (4)
SLOT_BATCHES = (2, 1, 1)    # batches per weight slot
NSLOT = len(SLOT_BATCHES)
N_WARM = 12

_COMPILED = None


def _build():
    nc = bacc.Bacc("TRN2", target_bir_lowering=False, debug=False)
    f32 = mybir.dt.float32
    f16 = mybir.dt.float16

    # xt: host-pre-transposed to [batch, partition, kt, m] so DMA lines are
    # contiguous per partition.
    xt_ap = nc.dram_tensor("xt", [BPC, P, KT, S], f16, kind="ExternalInput").ap()
    w_ap = nc.dram_tensor("w", [NSLOT, K, H], f16, kind="ExternalInput").ap()
    bias_ap = nc.dram_tensor("bias", [1, NSLOT * H], f32, kind="ExternalInput").ap()
    out_ap = nc.dram_tensor("out", [BPC, S, H], f16, kind="ExternalOutput").ap()

    with tile.TileContext(nc) as tc:
        with (
            tc.tile_pool(name="xt_pool", bufs=16) as xt_pool,
            tc.tile_pool(name="w_pool", bufs=16) as w_pool,
            tc.tile_pool(name="bias_pool", bufs=1) as bias_pool,
            tc.tile_pool(name="out_pool", bufs=6) as out_pool,
            tc.tile_pool(name="warm_pool", bufs=1) as warm_pool,
            tc.tile_pool(name="ps_pool", bufs=8, space="PSUM") as ps_pool,
        ):
            # bias row: all 3 slots' biases in one partition line (48 KB).
            bias_row = bias_pool.tile([1, NSLOT * H], f32, name="bias_row")
            nc.gpsimd.dma_start(bias_row[:], bias_ap[:])

            # Warm up the PE (HAM un-throttle) while the first DMAs land:
            # serialized same-bank matmuls pace themselves at ~0.43 us each.
            warm_x = warm_pool.tile([P, P], f16, name="warm_x", tag="wx")
            warm_w = warm_pool.tile([P, 512], f16, tag="ww", name="warm_w")
            nc.vector.memset(warm_x[:], 0.0)
            nc.vector.memset(warm_w[:], 0.0)
            warm_ps = ps_pool.tile([P, 512], f32, tag="ps", name="warm_ps")
            for _ in range(N_WARM):
                nc.tensor.matmul(
                    warm_ps[:], warm_x[:], warm_w[:], start=True, stop=True,
                    skip_group_check=True,
                )
            warm_out = out_pool.tile([P, 4], f32, name="warm_out", tag="warmo")
            nc.vector.tensor_copy(warm_out[:], warm_ps[:, 0:4])
            for _ in range(4):  # HAM keepalive over arrival jitter
                nc.tensor.ldweights(warm_x[:])

            def new_xt():
                return xt_pool.tile([P, KTC, S], f16, name="xt_t", tag="xt")

            def new_w():
                return w_pool.tile([P, NH], f16, tag="w", name="w_t")

            bi0 = 0  # first batch index of this slot
            for s in range(NSLOT):
                nb = SLOT_BATCHES[s]
                w_r = w_ap[s].rearrange("(kt p) n -> p kt n", p=P)
                xt_ts = []
                w_h0 = None
                if s == 0:
                    # Pipe-fill issue order.  The sync ring consistently
                    # delivers first bytes earliest, so it carries the
                    # fill-critical tiles in CONSUMPTION order of the
                    # kt-outer pair phase: xt c0, W kt0, kt1, c1, kt2, kt3,
                    # c2, c3.  W kt4-7 lead the scalar ring (out-stores only
                    # join it ~25 us in).
                    xt_b0 = [new_xt() for _ in range(NXC)]
                    w_h0 = [new_w() for _ in range(KT)]

                    def ld_xt(c):
                        return nc.sync.dma_start(
                            xt_b0[c][:], xt_ap[0, :, c * KTC : (c + 1) * KTC, :]
                        )

                    def ld_w(kt, eng):
                        return eng.dma_start(w_h0[kt][:], w_r[:, kt, 0:NH])

                    ld_xt(0)
                    gate = ld_w(0, nc.sync)  # c0+kt0 = the 1 MB that gates MM#0
                    ld_w(1, nc.sync)
                    ld_xt(1)
                    ld_w(2, nc.sync)
                    ld_w(3, nc.sync)
                    ld_xt(2)
                    ld_xt(3)
                    # kt4-7 ride the scalar ring, but ONLY after the gating
                    # megabyte has landed -- while exactly one queue has
                    # work, it gets all 16 SDMA engines, so c0+kt0 arrive
                    # ~2x sooner and the PE starts ~5 us earlier.
                    for kt in range(4, KT):
                        wi = ld_w(kt, nc.scalar)
                        if kt == 4:
                            _add_dep_helper(
                                wi.ins, gate.ins, sync=True,
                                reason="keep early SDMA bandwidth on the fill gate",
                            )
                    xt_ts.append(xt_b0)
                # bias for this slot, broadcast across partitions ON-CHIP
                # (before the gpsimd xt loads: needed by the first evict).
                bias_t = bias_pool.tile([P, H], f32, name="bias_t", tag="bt")
                nc.gpsimd.partition_broadcast(
                    bias_t[:], bias_row[0:1, s * H : (s + 1) * H]
                )
                # xt for the remaining batches rides the gpsimd (SWDGE)
                # ring, off the out-store path (and behind the fill gate).
                for b in range(len(xt_ts), nb):
                    chunks = []
                    for c in range(NXC):
                        xt_t = new_xt()
                        di = nc.gpsimd.dma_start(
                            xt_t[:], xt_ap[bi0 + b, :, c * KTC : (c + 1) * KTC, :]
                        )
                        if s == 0 and c == 0:
                            _add_dep_helper(
                                di.ins, gate.ins, sync=True,
                                reason="keep early SDMA bandwidth on the fill gate",
                            )
                        chunks.append(xt_t)
                    xt_ts.append(chunks)
                for half in range(NHALF):
                    if s == 0 and half == 0:
                        w_tiles = w_h0
                    else:
                        w_tiles = []
                        for kt in range(KT):
                            w_t = new_w()
                            nc.sync.dma_start(
                                w_t[:], w_r[:, kt, half * NH : (half + 1) * NH]
                            )
                            w_tiles.append(w_t)
                    def evict(m, ps, split=False):
                        b, mm = divmod(m, MT)
                        dst = out_ap[
                            bi0 + b,
                            mm * P : (mm + 1) * P,
                            half * NH : (half + 1) * NH,
                        ]
                        if split:
                            # tail: per-n4 stores so the last DMA is small
                            for n4 in range(NMM):
                                out_s = out_pool.tile([P, 512], f16, tag="oS")
                                nc.vector.tensor_add(
                                    out_s[:],
                                    ps[n4][:],
                                    bias_t[
                                        :,
                                        half * NH + n4 * 512 : half * NH
                                        + (n4 + 1) * 512,
                                    ],
                                )
                                nc.sync.dma_start(
                                    dst[:, n4 * 512 : (n4 + 1) * 512], out_s[:]
                                )
                            return
                        out_t = out_pool.tile([P, NH], f16)
                        for n4 in range(NMM):
                            nc.vector.tensor_add(
                                out_t[:, n4 * 512 : (n4 + 1) * 512],
                                ps[n4][:],
                                bias_t[
                                    :,
                                    half * NH + n4 * 512 : half * NH + (n4 + 1) * 512,
                                ],
                            )
                        nc.scalar.dma_start(dst, out_t[:])

                    def mk_ps():
                        return [
                            ps_pool.tile([P, 512], f32, tag="ps", name="ps")
                            for _ in range(NMM)
                        ]

                    def mm_group(m, kt, ps):
                        b, mm = divmod(m, MT)
                        xt_t = xt_ts[b][kt // KTC]
                        lhsT = xt_t[:, kt % KTC, mm * P : (mm + 1) * P]
                        for n4 in range(NMM):
                            nc.tensor.matmul(
                                ps[n4][:],
                                lhsT,
                                w_tiles[kt][:, n4 * 512 : (n4 + 1) * 512],
                                start=(kt == 0),
                                stop=(kt == KT - 1),
                            )

                    last = s == NSLOT - 1 and half == NHALF - 1
                    if s == 0 and half == 0:
                        # Pipe-fill phase: m-tiles 0-1 in kt-OUTER order, so
                        # each arriving W k-tile feeds 8 matmuls (~1.7 us) --
                        # matching the HWDGE delivery rate instead of
                        # outrunning it (8 psum banks = exactly 2 m-tiles).
                        ps01 = [mk_ps(), mk_ps()]
                        for kt in range(KT):
                            if kt:
                                # DMA-paced phase: the PE idles ~3 us per
                                # kt-step waiting for W -- feed the HAM
                                # activity monitor so the clock gate stays
                                # at 8/8 (these loads are overwritten by
                                # each matmul group's own LDWEIGHTS).
                                for _ in range(3):
                                    nc.tensor.ldweights(warm_x[:])
                            for m in (0, 1):
                                mm_group(m, kt, ps01[m])
                        for m in (0, 1):
                            evict(m, ps01[m])
                        rest = range(2, nb * MT)
                    else:
                        rest = range(nb * MT)
                    for m in rest:
                        ps = mk_ps()
                        for kt in range(KT):
                            mm_group(m, kt, ps)
                        evict(m, ps, split=last and m == nb * MT - 1)
                bi0 += nb
    nc.compile()
    return nc


def _get_compiled():
    global _COMPILED
    if _COMPILED is None:
        _COMPILED = _build()
    return _COMPILED


def _pack(cat_ids):
    """Assign batches to cores with slot capacities [2,1,1] per core.

    Returns per-core (idx, slot_cats): idx = 4 batch indices ordered
    [pair0, pair1, single_b, single_c]; slot_cats = categories for the 3 slots.
    Always feasible: #disjoint same-cat pairs = (32 - #odd-count cats)/2 >= 8.
    """
    cat_ids = np.asarray(cat_ids)
    by_cat = {}
    for i, c in enumerate(cat_ids.tolist()):
        by_cat.setdefault(c, []).append(i)
    pairs = []
    singles = []
    for c, idxs in sorted(by_cat.items()):
        n = len(idxs)
        for j in range(n // 2):
            pairs.append((c, idxs[2 * j], idxs[2 * j + 1]))
        if n % 2:
            singles.append((c, idxs[-1]))
    assert len(pairs) >= N_CORES, "impossible: <8 same-cat pairs among 32 batches"
    core_pairs = pairs[:N_CORES]
    # leftovers: extra pairs flatten into singles
    for c, i, j in pairs[N_CORES:]:
        singles.append((c, i))
        singles.append((c, j))
    assert len(singles) == 2 * N_CORES
    cores = []
    for ci in range(N_CORES):
        c, i, j = core_pairs[ci]
        (cb, ib), (cc, ic) = singles[2 * ci], singles[2 * ci + 1]
        cores.append(([i, j, ib, ic], [c, cb, cc]))
    return cores


def _host_xt(xb):
    """x[b] (n_batch, S, K) f32 -> (n_batch, P, KT, S) f16 partition-major."""
    n = xb.shape[0]
    # [n, S, K] -> [n, S, KT, P] -> [n, P, KT, S]
    return np.ascontiguousarray(
        xb.reshape(n, S, KT, P).transpose(0, 3, 2, 1).astype(np.float16)
    )


def run_sharded(x, cat_ids, W, b, trace=False, **spmd_kwargs):
    """Shard, run on 8 cores, unshard. Returns (out, BassKernelResults)."""
    x = np.ascontiguousarray(np.asarray(x), dtype=np.float32)
    cat_ids = np.asarray(cat_ids).astype(np.int64)
    W = np.ascontiguousarray(np.asarray(W), dtype=np.float32)
    b = np.ascontiguousarray(np.asarray(b), dtype=np.float32)

    nc = _get_compiled()
    cores = _pack(cat_ids)

    in_maps = []
    for idx, slot_cats in cores:
        in_maps.append(
            {
                "xt": _host_xt(x[idx]),
                "w": np.ascontiguousarray(W[slot_cats].astype(np.float16)),
                "bias": np.ascontiguousarray(b[slot_cats].reshape(1, -1)),
            }
        )

    res = run_bass_kernel_spmd(
        nc, in_maps, list(range(N_CORES)), trace=trace, **spmd_kwargs
    )

    out = np.empty((B, S, H), dtype=np.float32)
    for c, (idx, _) in enumerate(cores):
        out[idx] = res.results[c]["out"].astype(np.float32)
    return out, res


def kernel(x, cat_ids, W, b):
    out, _ = run_sharded(x, cat_ids, W, b)
    return out
